# revision 1
# baseline (speedup 1.0000x reference)
"""Fused LayerNorm -> Linear(H->I) -> GELU(erf) kernel for Trainium2.

Strategy: pure data parallelism over the 16384 (B*S) token rows across the
8 NeuronCores. Each core runs an identical (SPMD) Bass/Tile program on a
2048-row slice.

Per-core structure (W stays resident in SBUF, read once from HBM):
  - m-tiles (128 tokens) are processed in half-slabs of 8.
  - Per m-tile: DMA x [128,1024]; bn_stats/bn_aggr -> mean/var;
    rstd = 1/sqrt(var+eps) (ACT sqrt + DVE reciprocal); DVE applies
    (x-mu)*rstd; PE transposes the 128x128 blocks to [h,m] layout;
    DVE copies PSUM->SBUF rounding to float32r.
  - For nontrivial gamma/beta the post-transpose copy becomes a DVE
    tensor_scalar (gamma/beta are per-partition in [h,m] layout); for
    nonzero b a broadcast bias tile is added before gelu. Both are
    skipped for the trivial values this problem ships.
  - Matmuls run groups-outer over the output dim: all 8 m-tiles of a slab
    multiply against W group g before moving to g+1, so only the first
    4MB of W gates the pipeline start.
  - float32r streams 1 elem/cell/cycle on the PE (4x faster than fp32,
    ~1.4e-4 max rel err at K=1024 vs 2.5e-3 for bf16).
  - ACT applies erf-Gelu (LUT exact to ~2e-6) straight out of PSUM.
"""

import sys

if "/opt/trn_rl_repo" not in sys.path:
    sys.path.insert(0, "/opt/trn_rl_repo")

from contextlib import ExitStack

import numpy as np

import concourse.bass as bass
import concourse.tile as tile
from concourse import bacc, mybir
from concourse.masks import make_identity
from concourse.tile_rust import add_dep_helper

F32 = mybir.dt.float32
MM_DT = mybir.dt.float32r
LN_EPS = 1e-7
P = 128
N_CORES = 8


def build_program(m_loc, H, I, apply_gb, use_bias, act_func=None):
    """Build the per-core SPMD Bass program.

    m_loc: rows (tokens) per core. H: hidden (contraction) dim. I: output dim.
    apply_gb: apply gamma/beta on the post-transpose copies (False when
    gamma is all-ones and beta all-zeros: plain copy).
    use_bias: add b to the matmul output before gelu (False when b == 0).
    act_func: final activation (default Gelu; CoreSim tests pass Identity).
    """
    if act_func is None:
        act_func = mybir.ActivationFunctionType.Gelu
    KT = H // P          # k (contraction) tiles
    MT = m_loc // P      # token tiles per core
    MH = min(8, MT)      # m-tiles per half-slab
    NG = 4               # output-column groups
    NW = I // NG         # columns per group (1024 = 2 psum banks)
    NB = NW // 512       # psum-bank-sized matmuls per group per k

    nc = bacc.Bacc()
    x_h = nc.dram_tensor("x", [m_loc, H], F32, kind="ExternalInput")
    w_h = nc.dram_tensor("w", [H, I], F32, kind="ExternalInput")
    gam_h = nc.dram_tensor("gamma", [H], F32, kind="ExternalInput")
    bet_h = nc.dram_tensor("beta", [H], F32, kind="ExternalInput")
    if use_bias:
        b_h = nc.dram_tensor("b", [I], F32, kind="ExternalInput")
    y_h = nc.dram_tensor("y", [m_loc, I], F32, kind="ExternalOutput")

    with ExitStack() as ctx:
        tc = ctx.enter_context(tile.TileContext(nc))
        lean = apply_gb or use_bias  # general path: tighter SBUF budget
        consts = ctx.enter_context(tc.tile_pool(name="consts", bufs=1))
        xpool = ctx.enter_context(tc.tile_pool(name="xpool", bufs=2 if lean else 3))
        xtp = ctx.enter_context(
            tc.tile_pool(name="xtp", bufs=(max(2, MH - 2) if lean else MH + 2))
        )
        stats = ctx.enter_context(tc.tile_pool(name="stats", bufs=4))
        opool = ctx.enter_context(tc.tile_pool(name="opool", bufs=2))
        tpp = ctx.enter_context(tc.tile_pool(name="tpp", bufs=4, space="PSUM"))
        mmp = ctx.enter_context(tc.tile_pool(name="mmp", bufs=2, space="PSUM"))

        # W tiles allocated upfront; group 0 is DMA'd immediately (it gates
        # the first matmuls), later groups are deferred + dependency-gated
        # so their HBM traffic doesn't starve the x-tile loads.
        w_sb = {}
        for g in range(NG):
            for k in range(KT):
                w_sb[k, g] = consts.tile(
                    [P, NW], MM_DT, tag=f"w_{k}_{g}", name=f"w_{k}_{g}"
                )

        def emit_w_chunk(g, k, gate_inst=None):
            # float32r rounding happens inside the SWDGE cast-DMA
            dma = nc.gpsimd.dma_start(
                out=w_sb[k, g],
                in_=w_h[k * P:(k + 1) * P, g * NW:(g + 1) * NW],
            )
            if gate_inst is not None:
                add_dep_helper(
                    dma.ins, gate_inst,
                    reason="defer W chunk DMA to smooth HBM demand",
                )

        def emit_w_dmas(g, gate_inst=None):
            for k in range(KT):
                emit_w_chunk(g, k, gate_inst)

        ident = consts.tile([P, P], F32, tag="ident", name="ident")
        make_identity(nc, ident)

        # first 3 W chunks immediately; the rest of group 0 is gated on m0's
        # stats (x0 landed) so the x0/x1 loads get HBM priority while the
        # gated chunks still land before the first group's k>=3 matmuls
        for k in range(min(3, KT)):
            emit_w_chunk(0, k)

        # PE warm-up: throwaway matmuls so the HAM clock-gate opens
        # before the first real transposes/matmuls arrive
        warm_ps = tpp.tile([P, 4 * P], F32, tag="tp", name="warm_ps")
        for wi in range(8):
            nc.tensor.matmul(
                warm_ps[:, 0:P], lhsT=ident, rhs=ident, start=True, stop=True,
            )

        eps_t = consts.tile([P, 1], F32, tag="eps", name="eps")
        nc.vector.memset(eps_t, LN_EPS)

        gam_t = bet_t = None
        if apply_gb:
            # gamma/beta as [P, KT]: column k holds the k*128.. block
            gam_t = consts.tile([P, KT], F32, tag="gam", name="gam")
            bet_t = consts.tile([P, KT], F32, tag="bet", name="bet")
            nc.sync.dma_start(out=gam_t, in_=gam_h[:].rearrange("(k p) -> p k", p=P))
            nc.sync.dma_start(out=bet_t, in_=bet_h[:].rearrange("(k p) -> p k", p=P))

        # Bias vector [128, I]: broadcast of b across partitions
        b_bc = None
        if use_bias:
            b_bc = consts.tile([P, I], F32, tag="b_bc", name="b_bc")
            b_ap = b_h[:]
            nc.gpsimd.dma_start(
                out=b_bc,
                in_=bass.AP(
                    tensor=b_ap.tensor, offset=b_ap.offset,
                    ap=[[0, P]] + list(b_ap.ap),
                ),
            )

        def emit_mm_group(xT, m, g):
            ps = mmp.tile([P, NW], F32, tag="mm", name=f"mm_{m}_{g}")
            first_mm = None
            for k in range(KT):
                for h2 in range(NB):
                    mm = nc.tensor.matmul(
                        ps[:, h2 * 512:(h2 + 1) * 512],
                        lhsT=xT[:, k, :],
                        rhs=w_sb[k, g][:, h2 * 512:(h2 + 1) * 512],
                        start=(k == 0), stop=(k == KT - 1),
                    )
                    if first_mm is None:
                        first_mm = mm
            ot = opool.tile([P, NW], F32, tag="out", name=f"out_{m}_{g}")
            if use_bias:
                nc.vector.tensor_add(
                    out=ot, in0=ps, in1=b_bc[:, g * NW:(g + 1) * NW]
                )
                nc.scalar.activation(out=ot, in_=ot, func=act_func)
            else:
                nc.scalar.activation(out=ot, in_=ps, func=act_func)
            nc.sync.dma_start(
                out=y_h[m * P:(m + 1) * P, g * NW:(g + 1) * NW], in_=ot
            )
            return first_mm.ins

        assert MT % MH == 0
        prev_apply = None
        nst = H // 512
        x_tiles = {}

        x_dma_insts = {}

        def load_x(m):
            xt = xpool.tile([P, H], F32, tag="x", name=f"x_{m}")
            dmas = []
            for s in range(nst):
                dmas.append(nc.sync.dma_start(
                    out=xt[:, s * 512:(s + 1) * 512],
                    in_=x_h[m * P:(m + 1) * P, s * 512:(s + 1) * 512],
                ))
            x_dma_insts[m] = dmas
            x_tiles[m] = xt

        for half in range(MT // MH):
            ms = [half * MH + j for j in range(MH)]
            xT_tiles = {}
            for m in ms:
                if m not in x_tiles:
                    load_x(m)
                xt = x_tiles.pop(m)

                # LayerNorm stats (each chunk starts as its half arrives)
                st = stats.tile([P, nst, 6], F32, tag="bnst", name=f"bnst_{m}")
                stats_insts = []
                for s in range(nst):
                    stats_insts.append(nc.vector.bn_stats(
                        out=st[:, s, :], in_=xt[:, s * 512:(s + 1) * 512]
                    ))
                if prev_apply is not None:
                    # keep the DVE queue in m order: a DMA-gated later
                    # m-tile's stats must not head-block this one's apply
                    add_dep_helper(
                        stats_insts[0].ins, prev_apply,
                        reason="serialize LN chain in m order",
                    )
                if m == 0:
                    for k in range(min(3, KT), KT):
                        emit_w_chunk(0, k, gate_inst=stats_insts[0].ins)
                mv = stats.tile([P, 2], F32, tag="mv", name=f"mv_{m}")
                nc.vector.bn_aggr(out=mv, in_=st)
                rstd = stats.tile([P, 1], F32, tag="rstd", name=f"rstd_{m}")
                nc.scalar.activation(
                    out=rstd, in_=mv[:, 1:2],
                    func=mybir.ActivationFunctionType.Sqrt,
                    bias=eps_t, scale=1.0,
                )
                nc.vector.reciprocal(out=rstd, in_=rstd)
                # xc = (x - mu) * rstd  (in place)
                apply_inst = nc.vector.tensor_scalar(
                    out=xt, in0=xt, scalar1=mv[:, 0:1], scalar2=rstd,
                    op0=mybir.AluOpType.subtract, op1=mybir.AluOpType.mult,
                )
                prev_apply = apply_inst.ins

                # PE-transpose to [h, m]; DVE copies PSUM->SBUF (f32r round)
                xT = xtp.tile([P, KT, P], MM_DT, tag="xT", name=f"xT_{m}")
                for hb in range(KT // 4):
                    tp = tpp.tile([P, 4 * P], F32, tag="tp", name=f"tp_{m}_{hb}")
                    for j in range(4):
                        k = hb * 4 + j
                        nc.tensor.transpose(
                            out=tp[:, j * P:(j + 1) * P],
                            in_=xt[:, k * P:(k + 1) * P],
                            identity=ident,
                        )
                    if apply_gb:
                        for j in range(4):
                            k = hb * 4 + j
                            nc.vector.tensor_scalar(
                                out=xT[:, k, :], in0=tp[:, j * P:(j + 1) * P],
                                scalar1=gam_t[:, k:k + 1],
                                scalar2=bet_t[:, k:k + 1],
                                op0=mybir.AluOpType.mult,
                                op1=mybir.AluOpType.add,
                            )
                    else:
                        nc.vector.tensor_copy(
                            out=xT[:, hb * 4:(hb + 1) * 4, :], in_=tp
                        )
                xT_tiles[m] = xT
                # group-0 matmuls interleave with the next m-tile's LN
                fmm = emit_mm_group(xT, m, 0)
                if half == 0 and NG > 1:
                    # trickle group-1 W chunks gated on later g0 groups so
                    # they arrive shortly before the g1 phase needs them
                    mi0 = ms.index(m)
                    for k in range(KT):
                        if min(k // 2 + 4, MH - 1) == mi0:
                            emit_w_chunk(1, k, gate_inst=fmm)

            for g in range(1, NG):
                for mi, m in enumerate(ms):
                    fmm = emit_mm_group(xT_tiles[m], m, g)
                    if half == 0 and g + 1 < NG:
                        for k in range(mi * KT // MH, (mi + 1) * KT // MH):
                            emit_w_chunk(g + 1, k, gate_inst=fmm)
                    # prefetch the next slab's first x tiles during g2 so
                    # their DMAs issue (and land) before the slab boundary
                    if g == NG - 2 and mi < 3 and half + 1 < MT // MH:
                        load_x((half + 1) * MH + mi)

    return nc


def _run(hidden_states, ln_gamma, ln_beta, W, b, trace=False):
    from concourse.bass_utils import run_bass_kernel_spmd

    x = np.ascontiguousarray(np.asarray(hidden_states, dtype=np.float32))
    shp = x.shape
    H = shp[-1]
    x2 = x.reshape(-1, H)
    M = x2.shape[0]
    I = W.shape[1]
    assert M % (N_CORES * P) == 0
    m_loc = M // N_CORES

    W_np = np.ascontiguousarray(np.asarray(W, dtype=np.float32))
    g_np = np.ascontiguousarray(np.asarray(ln_gamma, dtype=np.float32))
    be_np = np.ascontiguousarray(np.asarray(ln_beta, dtype=np.float32))
    b_np = np.ascontiguousarray(np.asarray(b, dtype=np.float32))
    apply_gb = (not bool(np.all(g_np == 1.0))) or bool(np.any(be_np != 0.0))
    use_bias = bool(np.any(b_np != 0.0))

    nc = build_program(m_loc, H, I, apply_gb, use_bias)
    if not nc.is_finalized():
        nc.finalize()

    in_maps = []
    for c in range(N_CORES):
        im = {
            "x": np.ascontiguousarray(x2[c * m_loc:(c + 1) * m_loc]),
            "w": W_np,
            "gamma": g_np,
            "beta": be_np,
        }
        if use_bias:
            im["b"] = b_np
        in_maps.append(im)

    res = run_bass_kernel_spmd(
        nc, in_maps, core_ids=list(range(N_CORES)), trace=trace
    )
    y = np.concatenate([r["y"] for r in res.results], axis=0)
    y = y.reshape(shp[:-1] + (I,)).astype(np.float32)
    return y, res


def kernel(hidden_states, ln_gamma, ln_beta, W, b):
    y, _ = _run(hidden_states, ln_gamma, ln_beta, W, b, trace=False)
    return y

